# revision 56
# baseline (speedup 1.0000x reference)
"""nn_AdapFilter3d Trainium2 kernel — 8-core SPMD (data-parallel over (B,C)).

out[b,c,z,y,x] = sum_{i,j,k} pad(input)[b,c,z+i-1,y+j-1,x+k-1] * F[b,c,z,y,x,i,j,k]

Final strategy (per NeuronCore: 4 of the 32 (b,c) slices = 2 slice-pairs;
partitions p = 64*s + y; free dims carry (z, x) densely):

  - y-shift via accumulating matmuls with shift stationaries S_j (host
    pre-shifts F by -dy per j); x/z shifts are free-dim offsets into one
    padded dense (z,x) row per y-partition. 27 taps -> 9 slots (3 i-taps
    each; j,k fixed per slot).
  - Mixed-precision F split tuned to balance DMA / DVE / ScalarE / PE:
    5 slots stream as fp8 E3M4 (4-mantissa-bit, RMS 1.3%) and are
    upconverted fp8->bf16 by ScalarE (427ns/tap), 4 slots stream from
    HBM directly as bf16 (no engine cost, spends spare DMA bandwidth).
    All 27 DVE multiplies then run bf16 2x mode (278ns/tap).
    HBM/core: 8.8MB fp8-F + 9.4MB bf16-F + 2.2MB x + 1MB out ~ 21.5MB.
    Measured rel err 9.7e-3 (gate 2e-2).
  - DVE 2x needs even element bases: k=1 windows are odd, served via a
    +1-shifted second x copy built ON-CHIP by idle DVE copy cycles
    (saves 1.11MB of HBM read vs re-reading the x buffer at offset 1).
  - Engine-activity lessons (measured): gpsimd tensor ops next to DVE
    cause a 6.8x DVE slowdown (SBUF-fabric contention) — gpsimd only
    carries output-DMA descriptors; pushing ScalarE past ~70% duty
    slows every engine ~20% (chip activity throttle / P0).
  - PE: 27 accumulating 512-col matmuls/chunk (216ns warm / 427 cold),
    not binding. Each dma_start costs 0.6-1.4us of ring issue time, so
    transfers are whole-tile except the finer-grained first chunk.

Measured on 8xTRN2 (neuron-profile, SPMD all 8 cores): 89.7-90.4us HW
exec vs 109.8-114.7us for the prior bf16 baseline (run variance ~±3us).
Edge tuning: chunk 0 streams its direct-bf16 slots per-slot with per-slot
multiplies (fill gate ~0.4MB); the last chunk interleaves matmul groups
between its upconvert-multiply instrs so only 3 matmuls trail the final
multiply.

Self-contained: hardcodes shapes from the problem spec.
"""

import time

import numpy as np

import bass_rust
import concourse.bacc as bacc
import concourse.tile as tile
from concourse import mybir
from concourse.bass_utils import run_bass_kernel_spmd

B, C, D, H, W = 2, 16, 32, 64, 64
BC = B * C
TAPS = 27
N_CORES = 8
S = BC // N_CORES  # 4 slices per core
PAIRS = S // 2  # 2
ZC = 8  # z planes per chunk
NCHUNK = D // ZC  # 4
FD = ZC * W  # 512
SFD = 3 * FD  # 1536 (one slot = 3 i-taps)
CFD = TAPS * FD  # 13824 (one chunk of F)
DW = D * W  # 2048 dense (z,x) elements per (slice, y)
FRONT = 65  # zero pad around the dense (z,x) block (>= W+1)
XPLEN = FRONT + DW + FRONT

# slot s holds taps (i=0..2, j=SLOT_J[s], k=SLOT_K[s]); slot 0 is a
# DVE-fused fp8 multiply (1x), slots 1-5 are ScalarE-upconverted (2x),
# slots 6-8 stream from HBM directly as bf16 (2x, no upconvert) — spends
# spare DMA bandwidth to cut both DVE-1x and ScalarE work
SLOT_J = [2, 0, 1, 2, 0, 1, 2, 0, 1]
SLOT_K = [1, 2, 2, 2, 0, 0, 0, 1, 1]
NF8 = 5  # slots in the fp8 stream (all ScalarE-upconverted)
NUP = 5  # ScalarE-upconverted slots (0..4 in the fp8 stream)
NDIR = 4  # direct-bf16 slots

F32 = mybir.dt.float32
IO_DT = mybir.dt.bfloat16
F8 = mybir.dt.float8e3


def _overlap_ap(tile_ap, start, dims):
    """AP on tile_ap's tensor at element offset `start` with custom free dims
    [[stride, num], ...] (keeps the tile's partition dim)."""
    return bass_rust.AP(tile_ap.tensor, start, [list(tile_ap.ap[0])] + dims)


def _build():
    nc = bacc.Bacc()
    x_ext = nc.declare_dram_parameter("input", [PAIRS, 128, XPLEN], IO_DT, isOutput=False)
    f_ext = nc.declare_dram_parameter(
        "F", [PAIRS, 128, NCHUNK * NF8 * SFD], F8, isOutput=False
    )
    f16_ext = nc.declare_dram_parameter(
        "F16", [PAIRS, 128, NCHUNK * NDIR * SFD], IO_DT, isOutput=False
    )
    s_ext = nc.declare_dram_parameter("stat", [128, 3 * 128], IO_DT, isOutput=False)
    out_ext = nc.declare_dram_parameter("out", [PAIRS, 128, NCHUNK * FD], IO_DT, isOutput=True)

    with tile.TileContext(nc) as tc:
        with (
            tc.tile_pool(name="const", bufs=1) as cpool,
            tc.tile_pool(name="xp", bufs=2) as xpool,
            tc.tile_pool(name="fp", bufs=3) as fpool,
            tc.tile_pool(name="fb", bufs=2) as fbpool,
            tc.tile_pool(name="prod", bufs=3) as ppool,
            tc.tile_pool(name="osb", bufs=3) as opool,
            tc.tile_pool(name="ps", bufs=4, space="PSUM") as pspool,
        ):
            st = cpool.tile([128, 3 * 128], IO_DT)

            # x/x2/st on the scalar ring; F on the sync ring; out D2Ds on
            # the gpsimd ring. Pair-0 x/x2 split so chunk 0's window (first
            # ~650 elems) lands fast.
            # x2 (the +1-shifted copy for odd k=1 window bases) is built
            # ON-CHIP by idle DVE copy cycles instead of a second 1.11MB
            # HBM read — DMA is now the binding resource.
            XCUT = 1152
            xps, x2s = [], []
            for pair in range(PAIRS):
                xp = xpool.tile([128, XPLEN], IO_DT, tag="xp")
                x2 = xpool.tile([128, XPLEN - 1], IO_DT, tag="x2")
                if pair == 0:
                    nc.scalar.dma_start(xp[:, :XCUT], x_ext[pair, :, :XCUT])
                    nc.scalar.dma_start(st[:], s_ext[:])
                    nc.scalar.dma_start(xp[:, XCUT:], x_ext[pair, :, XCUT:])
                else:
                    nc.scalar.dma_start(xp[:, :], x_ext[pair, :, :])
                xps.append(xp)
                x2s.append(x2)
            # the first piece depends only on xp's first DMA piece; later
            # pieces are issued inside the loop so they never block the
            # first multiplies
            nc.vector.tensor_copy(x2s[0][:, : XCUT - 1], xps[0][:, 1:XCUT])

            for it in range(PAIRS * NCHUNK):
                pair, ch = divmod(it, NCHUNK)
                xp, x2 = xps[pair], x2s[pair]
                if it == 1:
                    nc.vector.tensor_copy(
                        x2[:, XCUT - 1 :], xp[:, XCUT:XPLEN]
                    )
                elif it == NCHUNK:
                    nc.vector.tensor_copy(x2[:, :], xp[:, 1:XPLEN])
                ft = fpool.tile([128, NF8 * SFD], F8, tag="ft")
                ftb = fpool.tile([128, NDIR * SFD], IO_DT, tag="ftb")
                base = ch * NF8 * SFD
                base16 = ch * NDIR * SFD
                first = it == 0
                last = it == PAIRS * NCHUNK - 1
                if first:
                    # stream chunk 0 at per-slot granularity for fast fill
                    nc.sync.dma_start(
                        ftb[:, :SFD], f16_ext[pair, :, base16 : base16 + SFD]
                    )
                    nc.sync.dma_start(
                        ft[:, : 2 * SFD], f_ext[pair, :, base : base + 2 * SFD]
                    )
                    nc.sync.dma_start(
                        ftb[:, SFD : 2 * SFD],
                        f16_ext[pair, :, base16 + SFD : base16 + 2 * SFD],
                    )
                    nc.sync.dma_start(
                        ft[:, 2 * SFD :], f_ext[pair, :, base + 2 * SFD : base + NF8 * SFD]
                    )
                    nc.sync.dma_start(
                        ftb[:, 2 * SFD : 3 * SFD],
                        f16_ext[pair, :, base16 + 2 * SFD : base16 + 3 * SFD],
                    )
                    nc.sync.dma_start(
                        ftb[:, 3 * SFD :],
                        f16_ext[pair, :, base16 + 3 * SFD : base16 + NDIR * SFD],
                    )
                else:
                    # ft first: its consumer chain (ScalarE copy -> DVE up)
                    # is ~6.7us longer than ftb's (DVE dir directly)
                    nc.sync.dma_start(
                        ft[:, :], f_ext[pair, :, base : base + NF8 * SFD]
                    )
                    nc.sync.dma_start(
                        ftb[:, :], f16_ext[pair, :, base16 : base16 + NDIR * SFD]
                    )

                fb = fbpool.tile([128, NUP * SFD], IO_DT, tag="fb")
                if first:
                    nc.scalar.copy(fb[:, : 2 * SFD], ft[:, : 2 * SFD])
                    nc.scalar.copy(fb[:, 2 * SFD :], ft[:, 2 * SFD :])
                else:
                    nc.scalar.copy(fb[:, :], ft[:, :])

                prod = ppool.tile([128, CFD], IO_DT, tag="prod")
                psum = pspool.tile([128, FD], F32, tag="ps")
                xb = ch * FD  # even window base; +k for k in {0,2} on xp, x2 for k=1

                def slot_aps(t, s0, n):
                    return t[:, s0 * SFD : (s0 + n) * SFD].rearrange(
                        "p (s i e) -> p s i e", s=n, i=3
                    )

                def slot_ap2(t, s):
                    return t[:, s * SFD : (s + 1) * SFD].rearrange(
                        "p (i e) -> p i e", i=3
                    )

                def mm(s, i, start=False, stop=False):
                    t = 3 * s + i
                    nc.tensor.matmul(
                        psum[:],
                        st[:, SLOT_J[s] * 128 : (SLOT_J[s] + 1) * 128],
                        prod[:, t * FD : (t + 1) * FD],
                        start=start,
                        stop=stop,
                    )

                def dir_muls():
                    # direct-bf16 slots: 5-6 (k=0) merged; 7-8 (k=1 via x2)
                    nc.vector.tensor_mul(
                        slot_aps(prod, 5, 2),
                        _overlap_ap(xp[:], xb, [[0, 2], [W, 3], [1, FD]]),
                        ftb[:, : 2 * SFD].rearrange("p (s i e) -> p s i e", s=2, i=3),
                    )
                    nc.vector.tensor_mul(
                        slot_aps(prod, 7, 2),
                        _overlap_ap(x2[:], xb, [[0, 2], [W, 3], [1, FD]]),
                        ftb[:, 2 * SFD :].rearrange("p (s i e) -> p s i e", s=2, i=3),
                    )

                def up_muls():
                    # slot0 (k=1 via x2); slots1-3 (k=2) merged; slot4 (k=0)
                    nc.vector.tensor_mul(
                        slot_ap2(prod, 0),
                        _overlap_ap(x2[:], xb, [[W, 3], [1, FD]]),
                        fb[:, :SFD].rearrange("p (i e) -> p i e", i=3),
                    )
                    nc.vector.tensor_mul(
                        slot_aps(prod, 1, 3),
                        _overlap_ap(xp[:], xb + 2, [[0, 3], [W, 3], [1, FD]]),
                        fb[:, SFD : 4 * SFD].rearrange(
                            "p (s i e) -> p s i e", s=3, i=3
                        ),
                    )
                    nc.vector.tensor_mul(
                        slot_ap2(prod, 4),
                        _overlap_ap(xp[:], xb, [[W, 3], [1, FD]]),
                        fb[:, 4 * SFD :].rearrange("p (i e) -> p i e", i=3),
                    )

                def dir_mul_slot(s):
                    src = x2 if SLOT_K[s] == 1 else xp
                    nc.vector.tensor_mul(
                        slot_ap2(prod, s),
                        _overlap_ap(src[:], xb, [[W, 3], [1, FD]]),
                        ftb[:, (s - 5) * SFD : (s - 4) * SFD].rearrange(
                            "p (i e) -> p i e", i=3
                        ),
                    )

                if first:
                    # per-slot dir stream so the first multiply waits for
                    # only ~0.4MB instead of 0.8MB
                    for s in (5, 6, 7, 8):
                        dir_mul_slot(s)
                        mm(s, 0, start=(s == 5))
                        mm(s, 1)
                        mm(s, 2)
                else:
                    dir_muls()
                    for s in (5, 6, 7, 8):
                        mm(s, 0, start=(s == 5))
                        mm(s, 1)
                        mm(s, 2)
                if not last:
                    up_muls()
                    for s in (0, 1, 2, 3, 4):
                        mm(s, 0)
                        mm(s, 1)
                        mm(s, 2, stop=(s == 4))
                else:
                    # tail: interleave mm groups between the up-multiply
                    # instrs so only 3 matmuls queue after the last multiply
                    nc.vector.tensor_mul(
                        slot_ap2(prod, 0),
                        _overlap_ap(x2[:], xb, [[W, 3], [1, FD]]),
                        fb[:, :SFD].rearrange("p (i e) -> p i e", i=3),
                    )
                    mm(0, 0)
                    mm(0, 1)
                    mm(0, 2)
                    nc.vector.tensor_mul(
                        slot_aps(prod, 1, 3),
                        _overlap_ap(xp[:], xb + 2, [[0, 3], [W, 3], [1, FD]]),
                        fb[:, SFD : 4 * SFD].rearrange(
                            "p (s i e) -> p s i e", s=3, i=3
                        ),
                    )
                    for s in (1, 2, 3):
                        mm(s, 0)
                        mm(s, 1)
                        mm(s, 2)
                    nc.vector.tensor_mul(
                        slot_ap2(prod, 4),
                        _overlap_ap(xp[:], xb, [[W, 3], [1, FD]]),
                        fb[:, 4 * SFD :].rearrange("p (i e) -> p i e", i=3),
                    )
                    mm(4, 0)
                    mm(4, 1)
                    mm(4, 2, stop=True)
                osb = opool.tile([128, FD], IO_DT, tag="osb")
                nc.scalar.copy(osb[:], psum[:])
                nc.gpsimd.dma_start(
                    out_ext[pair, :, ch * FD : (ch + 1) * FD], osb[:]
                )
    nc.compile()
    return nc


_NC_CACHE = {}


def _host_inputs(input, F):
    """FULL inputs -> per-core in_maps with the kernel's layouts."""
    io_np = mybir.dt.np(IO_DT)
    f8_np = mybir.dt.np(F8)
    # x dense rows: xs[bc, y, FRONT + z*W + x]
    xs = np.zeros((BC, H, XPLEN), dtype=io_np)
    xs[:, :, FRONT : FRONT + DW] = (
        input.reshape(BC, D, H, W).transpose(0, 2, 1, 3).reshape(BC, H, DW).astype(io_np)
    )
    xs = xs.reshape(BC // 2, 128, XPLEN)

    # F pre-shifted along y by -dy per j, slot-ordered taps, edge taps zeroed
    base = np.ascontiguousarray(
        F.reshape(BC, D, H, W, 3, 3, 3).transpose(0, 2, 5, 4, 6, 1, 3)
    )  # [bc, y, j, i, k, z, x]
    Hs = np.zeros_like(base)
    Hs[:, : H - 1, 0] = base[:, 1:, 0]
    Hs[:, :, 1] = base[:, :, 1]
    Hs[:, 1:, 2] = base[:, : H - 1, 2]
    Hs[:, :, :, :, 0, :, 0] = 0
    Hs[:, :, :, :, 2, :, W - 1] = 0
    Hs[:, :, :, 0, :, 0, :] = 0
    Hs[:, :, :, 2, :, D - 1, :] = 0
    # slot-major: [bc, y, s, i, z, x]
    Hs = np.stack([Hs[:, :, SLOT_J[s], :, SLOT_K[s]] for s in range(9)], axis=2)
    fs = (
        Hs[:, :, :NF8]
        .reshape(BC, H, NF8, 3, NCHUNK, ZC, W)
        .transpose(0, 1, 4, 2, 3, 5, 6)  # [bc, y, ch, s, i, zc, x]
        .reshape(BC // 2, 128, NCHUNK * NF8 * SFD)
        .astype(f8_np)
    )
    fs16 = (
        Hs[:, :, NF8:]
        .reshape(BC, H, NDIR, 3, NCHUNK, ZC, W)
        .transpose(0, 1, 4, 2, 3, 5, 6)
        .reshape(BC // 2, 128, NCHUNK * NDIR * SFD)
        .astype(io_np)
    )

    # stationaries: st[kk, j*128+m] = 1 iff kk == m + (j-1), same 64-block
    stm = np.zeros((128, 3, 128), dtype=np.float32)
    for j in range(3):
        Sj = np.eye(128, k=-(j - 1), dtype=np.float32)
        Sj[0:64, 64:128] = 0
        Sj[64:128, 0:64] = 0
        stm[:, j, :] = Sj
    stm = stm.reshape(128, 3 * 128).astype(io_np)

    return [
        {
            "input": xs[c * PAIRS : (c + 1) * PAIRS],
            "F": fs[c * PAIRS : (c + 1) * PAIRS],
            "F16": fs16[c * PAIRS : (c + 1) * PAIRS],
            "stat": stm,
        }
        for c in range(N_CORES)
    ]


def kernel(input: np.ndarray, F: np.ndarray) -> np.ndarray:
    input = np.asarray(input)
    F = np.asarray(F)
    assert input.shape == (B, C, D, H, W), input.shape
    assert F.shape == (B, C, D, H, W, 3, 3, 3), F.shape

    if "nc" not in _NC_CACHE:
        _NC_CACHE["nc"] = _build()
    nc = _NC_CACHE["nc"]

    in_maps = _host_inputs(input, F)
    # the fleet occasionally throws transient NRT_EXEC_UNIT_UNRECOVERABLE
    # device errors (observed in dev, cleared on retry)
    last_err = None
    out = None
    for _attempt in range(4):
        try:
            res = run_bass_kernel_spmd(nc, in_maps, core_ids=list(range(N_CORES)))
        except Exception as e:  # noqa: BLE001
            last_err = e
            time.sleep(2.0)
            continue
        out = np.concatenate(
            [
                np.asarray(res.results[c]["out"], dtype=np.float32)
                for c in range(N_CORES)
            ],
            axis=0,
        )  # [BC/2, 128, NCHUNK*FD]
        if np.isfinite(out).all():
            break
        last_err = RuntimeError("non-finite output (transient device flake)")
    else:
        raise last_err
    out = (
        out.reshape(BC // 2, 2, H, NCHUNK, ZC, W)
        .transpose(0, 1, 3, 4, 2, 5)  # [pair, s, ch, zc, y, x]
        .reshape(B, C, D, H, W)
        .astype(np.float32)
    )
    return np.ascontiguousarray(out)


# revision 58
# speedup vs baseline: 1.0008x; 1.0008x over previous
"""nn_AdapFilter3d Trainium2 kernel — 8-core SPMD (data-parallel over (B,C)).

out[b,c,z,y,x] = sum_{i,j,k} pad(input)[b,c,z+i-1,y+j-1,x+k-1] * F[b,c,z,y,x,i,j,k]

Final strategy (per NeuronCore: 4 of the 32 (b,c) slices = 2 slice-pairs;
partitions p = 64*s + y; free dims carry (z, x) densely):

  - y-shift via accumulating matmuls with shift stationaries S_j (host
    pre-shifts F by -dy per j); x/z shifts are free-dim offsets into one
    padded dense (z,x) row per y-partition. 27 taps -> 9 slots (3 i-taps
    each; j,k fixed per slot).
  - Mixed-precision F split tuned to balance DMA / DVE / ScalarE / PE:
    5 slots stream as fp8 E3M4 (4-mantissa-bit, RMS 1.3%) and are
    upconverted fp8->bf16 by ScalarE (427ns/tap), 4 slots stream from
    HBM directly as bf16 (no engine cost, spends spare DMA bandwidth).
    All 27 DVE multiplies then run bf16 2x mode (278ns/tap).
    HBM/core: 8.8MB fp8-F + 9.4MB bf16-F + 2.2MB x + 1MB out ~ 21.5MB.
    Measured rel err 9.7e-3 (gate 2e-2).
  - DVE 2x needs even element bases: k=1 windows are odd, served via a
    +1-shifted second x copy built ON-CHIP by idle DVE copy cycles
    (saves 1.11MB of HBM read vs re-reading the x buffer at offset 1).
  - Engine-activity lessons (measured): gpsimd tensor ops next to DVE
    cause a 6.8x DVE slowdown (SBUF-fabric contention) — gpsimd only
    carries output-DMA descriptors; pushing ScalarE past ~70% duty
    slows every engine ~20% (chip activity throttle / P0).
  - PE: 27 accumulating 512-col matmuls/chunk (216ns warm / 427 cold),
    not binding. Each dma_start costs 0.6-1.4us of ring issue time, so
    transfers are whole-tile except the finer-grained first chunk.

Measured on 8xTRN2 (neuron-profile, SPMD all 8 cores): 86.2-90.3us HW
exec vs 109.8-114.7us for the prior bf16 baseline (run variance ~±3us).
Edge tuning: chunk 0 streams its direct-bf16 slots per-slot with per-slot
multiplies (fill gate ~0.4MB); the last chunk interleaves matmul groups
between its upconvert-multiply instrs so only 3 matmuls trail the final
multiply.

Self-contained: hardcodes shapes from the problem spec.
"""

import time

import numpy as np

import bass_rust
import concourse.bacc as bacc
import concourse.tile as tile
from concourse import mybir
from concourse.bass_utils import run_bass_kernel_spmd

B, C, D, H, W = 2, 16, 32, 64, 64
BC = B * C
TAPS = 27
N_CORES = 8
S = BC // N_CORES  # 4 slices per core
PAIRS = S // 2  # 2
ZC = 8  # z planes per chunk
NCHUNK = D // ZC  # 4
FD = ZC * W  # 512
SFD = 3 * FD  # 1536 (one slot = 3 i-taps)
CFD = TAPS * FD  # 13824 (one chunk of F)
DW = D * W  # 2048 dense (z,x) elements per (slice, y)
FRONT = 65  # zero pad around the dense (z,x) block (>= W+1)
XPLEN = FRONT + DW + FRONT

# slot s holds taps (i=0..2, j=SLOT_J[s], k=SLOT_K[s]); slot 0 is a
# DVE-fused fp8 multiply (1x), slots 1-5 are ScalarE-upconverted (2x),
# slots 6-8 stream from HBM directly as bf16 (2x, no upconvert) — spends
# spare DMA bandwidth to cut both DVE-1x and ScalarE work
SLOT_J = [2, 0, 1, 2, 0, 1, 2, 0, 1]
SLOT_K = [1, 2, 2, 2, 0, 0, 0, 1, 1]
NF8 = 5  # slots in the fp8 stream (all ScalarE-upconverted)
NUP = 5  # ScalarE-upconverted slots (0..4 in the fp8 stream)
NDIR = 4  # direct-bf16 slots

F32 = mybir.dt.float32
IO_DT = mybir.dt.bfloat16
F8 = mybir.dt.float8e3


def _overlap_ap(tile_ap, start, dims):
    """AP on tile_ap's tensor at element offset `start` with custom free dims
    [[stride, num], ...] (keeps the tile's partition dim)."""
    return bass_rust.AP(tile_ap.tensor, start, [list(tile_ap.ap[0])] + dims)


def _build():
    nc = bacc.Bacc()
    x_ext = nc.declare_dram_parameter("input", [PAIRS, 128, XPLEN], IO_DT, isOutput=False)
    f_ext = nc.declare_dram_parameter(
        "F", [PAIRS, 128, NCHUNK * NF8 * SFD], F8, isOutput=False
    )
    f16_ext = nc.declare_dram_parameter(
        "F16", [PAIRS, 128, NCHUNK * NDIR * SFD], IO_DT, isOutput=False
    )
    s_ext = nc.declare_dram_parameter("stat", [128, 3 * 128], IO_DT, isOutput=False)
    out_ext = nc.declare_dram_parameter("out", [PAIRS, 128, NCHUNK * FD], IO_DT, isOutput=True)

    with tile.TileContext(nc) as tc:
        with (
            tc.tile_pool(name="const", bufs=1) as cpool,
            tc.tile_pool(name="xp", bufs=2) as xpool,
            tc.tile_pool(name="fp", bufs=3) as fpool,
            tc.tile_pool(name="fb", bufs=2) as fbpool,
            tc.tile_pool(name="prod", bufs=3) as ppool,
            tc.tile_pool(name="osb", bufs=3) as opool,
            tc.tile_pool(name="ps", bufs=4, space="PSUM") as pspool,
        ):
            st = cpool.tile([128, 3 * 128], IO_DT)

            # x/x2/st on the scalar ring; F on the sync ring; out D2Ds on
            # the gpsimd ring. Pair-0 x/x2 split so chunk 0's window (first
            # ~650 elems) lands fast.
            # x2 (the +1-shifted copy for odd k=1 window bases) is built
            # ON-CHIP by idle DVE copy cycles instead of a second 1.11MB
            # HBM read — DMA is now the binding resource.
            XCUT = 1152
            xps, x2s = [], []
            for pair in range(PAIRS):
                xp = xpool.tile([128, XPLEN], IO_DT, tag="xp")
                x2 = xpool.tile([128, XPLEN - 1], IO_DT, tag="x2")
                if pair == 0:
                    nc.scalar.dma_start(xp[:, :XCUT], x_ext[pair, :, :XCUT])
                    nc.scalar.dma_start(st[:], s_ext[:])
                    nc.scalar.dma_start(xp[:, XCUT:], x_ext[pair, :, XCUT:])
                else:
                    nc.scalar.dma_start(xp[:, :], x_ext[pair, :, :])
                xps.append(xp)
                x2s.append(x2)
            # the first piece depends only on xp's first DMA piece; later
            # pieces are issued inside the loop so they never block the
            # first multiplies
            nc.vector.tensor_copy(x2s[0][:, : XCUT - 1], xps[0][:, 1:XCUT])

            for it in range(PAIRS * NCHUNK):
                pair, ch = divmod(it, NCHUNK)
                xp, x2 = xps[pair], x2s[pair]
                if it == 1:
                    nc.vector.tensor_copy(
                        x2[:, XCUT - 1 :], xp[:, XCUT:XPLEN]
                    )
                elif it == NCHUNK:
                    nc.vector.tensor_copy(x2[:, :], xp[:, 1:XPLEN])
                ft = fpool.tile([128, NF8 * SFD], F8, tag="ft")
                ftb = fpool.tile([128, NDIR * SFD], IO_DT, tag="ftb")
                base = ch * NF8 * SFD
                base16 = ch * NDIR * SFD
                first = it == 0
                last = it == PAIRS * NCHUNK - 1
                if first:
                    # stream chunk 0 at per-slot granularity for fast fill;
                    # fp8 pieces first (their ScalarE->DVE chain is longest)
                    nc.sync.dma_start(
                        ft[:, : 2 * SFD], f_ext[pair, :, base : base + 2 * SFD]
                    )
                    nc.sync.dma_start(
                        ftb[:, :SFD], f16_ext[pair, :, base16 : base16 + SFD]
                    )
                    nc.sync.dma_start(
                        ft[:, 2 * SFD :], f_ext[pair, :, base + 2 * SFD : base + NF8 * SFD]
                    )
                    nc.sync.dma_start(
                        ftb[:, SFD : 2 * SFD],
                        f16_ext[pair, :, base16 + SFD : base16 + 2 * SFD],
                    )
                    nc.sync.dma_start(
                        ftb[:, 2 * SFD : 3 * SFD],
                        f16_ext[pair, :, base16 + 2 * SFD : base16 + 3 * SFD],
                    )
                    nc.sync.dma_start(
                        ftb[:, 3 * SFD :],
                        f16_ext[pair, :, base16 + 3 * SFD : base16 + NDIR * SFD],
                    )
                else:
                    # ft first: its consumer chain (ScalarE copy -> DVE up)
                    # is ~6.7us longer than ftb's (DVE dir directly)
                    nc.sync.dma_start(
                        ft[:, :], f_ext[pair, :, base : base + NF8 * SFD]
                    )
                    nc.sync.dma_start(
                        ftb[:, :], f16_ext[pair, :, base16 : base16 + NDIR * SFD]
                    )

                fb = fbpool.tile([128, NUP * SFD], IO_DT, tag="fb")
                if first:
                    nc.scalar.copy(fb[:, : 2 * SFD], ft[:, : 2 * SFD])
                    nc.scalar.copy(fb[:, 2 * SFD :], ft[:, 2 * SFD :])
                else:
                    nc.scalar.copy(fb[:, :], ft[:, :])

                prod = ppool.tile([128, CFD], IO_DT, tag="prod")
                psum = pspool.tile([128, FD], F32, tag="ps")
                xb = ch * FD  # even window base; +k for k in {0,2} on xp, x2 for k=1

                def slot_aps(t, s0, n):
                    return t[:, s0 * SFD : (s0 + n) * SFD].rearrange(
                        "p (s i e) -> p s i e", s=n, i=3
                    )

                def slot_ap2(t, s):
                    return t[:, s * SFD : (s + 1) * SFD].rearrange(
                        "p (i e) -> p i e", i=3
                    )

                def mm(s, i, start=False, stop=False):
                    t = 3 * s + i
                    nc.tensor.matmul(
                        psum[:],
                        st[:, SLOT_J[s] * 128 : (SLOT_J[s] + 1) * 128],
                        prod[:, t * FD : (t + 1) * FD],
                        start=start,
                        stop=stop,
                    )

                def dir_muls():
                    # direct-bf16 slots: 5-6 (k=0) merged; 7-8 (k=1 via x2)
                    nc.vector.tensor_mul(
                        slot_aps(prod, 5, 2),
                        _overlap_ap(xp[:], xb, [[0, 2], [W, 3], [1, FD]]),
                        ftb[:, : 2 * SFD].rearrange("p (s i e) -> p s i e", s=2, i=3),
                    )
                    nc.vector.tensor_mul(
                        slot_aps(prod, 7, 2),
                        _overlap_ap(x2[:], xb, [[0, 2], [W, 3], [1, FD]]),
                        ftb[:, 2 * SFD :].rearrange("p (s i e) -> p s i e", s=2, i=3),
                    )

                def up_muls():
                    # slot0 (k=1 via x2); slots1-3 (k=2) merged; slot4 (k=0)
                    nc.vector.tensor_mul(
                        slot_ap2(prod, 0),
                        _overlap_ap(x2[:], xb, [[W, 3], [1, FD]]),
                        fb[:, :SFD].rearrange("p (i e) -> p i e", i=3),
                    )
                    nc.vector.tensor_mul(
                        slot_aps(prod, 1, 3),
                        _overlap_ap(xp[:], xb + 2, [[0, 3], [W, 3], [1, FD]]),
                        fb[:, SFD : 4 * SFD].rearrange(
                            "p (s i e) -> p s i e", s=3, i=3
                        ),
                    )
                    nc.vector.tensor_mul(
                        slot_ap2(prod, 4),
                        _overlap_ap(xp[:], xb, [[W, 3], [1, FD]]),
                        fb[:, 4 * SFD :].rearrange("p (i e) -> p i e", i=3),
                    )

                def dir_mul_slot(s):
                    src = x2 if SLOT_K[s] == 1 else xp
                    nc.vector.tensor_mul(
                        slot_ap2(prod, s),
                        _overlap_ap(src[:], xb, [[W, 3], [1, FD]]),
                        ftb[:, (s - 5) * SFD : (s - 4) * SFD].rearrange(
                            "p (i e) -> p i e", i=3
                        ),
                    )

                if first:
                    # per-slot dir stream so the first multiply waits for
                    # only ~0.4MB instead of 0.8MB
                    for s in (5, 6, 7, 8):
                        dir_mul_slot(s)
                        mm(s, 0, start=(s == 5))
                        mm(s, 1)
                        mm(s, 2)
                else:
                    dir_muls()
                    for s in (5, 6, 7, 8):
                        mm(s, 0, start=(s == 5))
                        mm(s, 1)
                        mm(s, 2)
                if not last:
                    up_muls()
                    for s in (0, 1, 2, 3, 4):
                        mm(s, 0)
                        mm(s, 1)
                        mm(s, 2, stop=(s == 4))
                else:
                    # tail: interleave mm groups between the up-multiply
                    # instrs so only 3 matmuls queue after the last multiply
                    nc.vector.tensor_mul(
                        slot_ap2(prod, 0),
                        _overlap_ap(x2[:], xb, [[W, 3], [1, FD]]),
                        fb[:, :SFD].rearrange("p (i e) -> p i e", i=3),
                    )
                    mm(0, 0)
                    mm(0, 1)
                    mm(0, 2)
                    nc.vector.tensor_mul(
                        slot_aps(prod, 1, 3),
                        _overlap_ap(xp[:], xb + 2, [[0, 3], [W, 3], [1, FD]]),
                        fb[:, SFD : 4 * SFD].rearrange(
                            "p (s i e) -> p s i e", s=3, i=3
                        ),
                    )
                    for s in (1, 2, 3):
                        mm(s, 0)
                        mm(s, 1)
                        mm(s, 2)
                    nc.vector.tensor_mul(
                        slot_ap2(prod, 4),
                        _overlap_ap(xp[:], xb, [[W, 3], [1, FD]]),
                        fb[:, 4 * SFD :].rearrange("p (i e) -> p i e", i=3),
                    )
                    mm(4, 0)
                    mm(4, 1)
                    mm(4, 2, stop=True)
                osb = opool.tile([128, FD], IO_DT, tag="osb")
                nc.scalar.copy(osb[:], psum[:])
                nc.gpsimd.dma_start(
                    out_ext[pair, :, ch * FD : (ch + 1) * FD], osb[:]
                )
    nc.compile()
    return nc


_NC_CACHE = {}


def _host_inputs(input, F):
    """FULL inputs -> per-core in_maps with the kernel's layouts."""
    io_np = mybir.dt.np(IO_DT)
    f8_np = mybir.dt.np(F8)
    # x dense rows: xs[bc, y, FRONT + z*W + x]
    xs = np.zeros((BC, H, XPLEN), dtype=io_np)
    xs[:, :, FRONT : FRONT + DW] = (
        input.reshape(BC, D, H, W).transpose(0, 2, 1, 3).reshape(BC, H, DW).astype(io_np)
    )
    xs = xs.reshape(BC // 2, 128, XPLEN)

    # F pre-shifted along y by -dy per j, slot-ordered taps, edge taps zeroed
    base = np.ascontiguousarray(
        F.reshape(BC, D, H, W, 3, 3, 3).transpose(0, 2, 5, 4, 6, 1, 3)
    )  # [bc, y, j, i, k, z, x]
    Hs = np.zeros_like(base)
    Hs[:, : H - 1, 0] = base[:, 1:, 0]
    Hs[:, :, 1] = base[:, :, 1]
    Hs[:, 1:, 2] = base[:, : H - 1, 2]
    Hs[:, :, :, :, 0, :, 0] = 0
    Hs[:, :, :, :, 2, :, W - 1] = 0
    Hs[:, :, :, 0, :, 0, :] = 0
    Hs[:, :, :, 2, :, D - 1, :] = 0
    # slot-major: [bc, y, s, i, z, x]
    Hs = np.stack([Hs[:, :, SLOT_J[s], :, SLOT_K[s]] for s in range(9)], axis=2)
    fs = (
        Hs[:, :, :NF8]
        .reshape(BC, H, NF8, 3, NCHUNK, ZC, W)
        .transpose(0, 1, 4, 2, 3, 5, 6)  # [bc, y, ch, s, i, zc, x]
        .reshape(BC // 2, 128, NCHUNK * NF8 * SFD)
        .astype(f8_np)
    )
    fs16 = (
        Hs[:, :, NF8:]
        .reshape(BC, H, NDIR, 3, NCHUNK, ZC, W)
        .transpose(0, 1, 4, 2, 3, 5, 6)
        .reshape(BC // 2, 128, NCHUNK * NDIR * SFD)
        .astype(io_np)
    )

    # stationaries: st[kk, j*128+m] = 1 iff kk == m + (j-1), same 64-block
    stm = np.zeros((128, 3, 128), dtype=np.float32)
    for j in range(3):
        Sj = np.eye(128, k=-(j - 1), dtype=np.float32)
        Sj[0:64, 64:128] = 0
        Sj[64:128, 0:64] = 0
        stm[:, j, :] = Sj
    stm = stm.reshape(128, 3 * 128).astype(io_np)

    return [
        {
            "input": xs[c * PAIRS : (c + 1) * PAIRS],
            "F": fs[c * PAIRS : (c + 1) * PAIRS],
            "F16": fs16[c * PAIRS : (c + 1) * PAIRS],
            "stat": stm,
        }
        for c in range(N_CORES)
    ]


def kernel(input: np.ndarray, F: np.ndarray) -> np.ndarray:
    input = np.asarray(input)
    F = np.asarray(F)
    assert input.shape == (B, C, D, H, W), input.shape
    assert F.shape == (B, C, D, H, W, 3, 3, 3), F.shape

    if "nc" not in _NC_CACHE:
        _NC_CACHE["nc"] = _build()
    nc = _NC_CACHE["nc"]

    in_maps = _host_inputs(input, F)
    # the fleet occasionally throws transient NRT_EXEC_UNIT_UNRECOVERABLE
    # device errors (observed in dev, cleared on retry)
    last_err = None
    out = None
    for _attempt in range(4):
        try:
            res = run_bass_kernel_spmd(nc, in_maps, core_ids=list(range(N_CORES)))
        except Exception as e:  # noqa: BLE001
            last_err = e
            time.sleep(2.0)
            continue
        out = np.concatenate(
            [
                np.asarray(res.results[c]["out"], dtype=np.float32)
                for c in range(N_CORES)
            ],
            axis=0,
        )  # [BC/2, 128, NCHUNK*FD]
        if np.isfinite(out).all():
            break
        last_err = RuntimeError("non-finite output (transient device flake)")
    else:
        raise last_err
    out = (
        out.reshape(BC // 2, 2, H, NCHUNK, ZC, W)
        .transpose(0, 1, 3, 4, 2, 5)  # [pair, s, ch, zc, y, x]
        .reshape(B, C, D, H, W)
        .astype(np.float32)
    )
    return np.ascontiguousarray(out)
